# revision 24
# baseline (speedup 1.0000x reference)
"""Trainium2 Bass kernel for nn_KeySelect (sparse_attention).

Sharding: 8 shards = (4 batches) x (2 spatial H-halves). Odd cores get a
vertically FLIPPED frame (slab rows, conv-tap dh order, attention kh order
all reversed on the host), so both halves share one SPMD program whose fake
interior boundary is always at the bottom. Validity then shrinks downward
only, and each stage computes exactly the rows it needs:
conv1 39, conv2 38, weighting 34, conv3/conv4 33, conv5 32 (vs 40 for every
stage in the unflipped scheme).

Layout: activation buffers are [C<=128 partitions, 48*65] -- rows of 65 (1
zero pad col + 64 data cols). A dw=+1 shifted read of row r's last column
lands on row r+1's pad column, so one shared pad per row suffices. A 3x3
conv is 9 PSUM-accumulated matmuls at free-dim offsets dh*65+dw. BN+ReLU is
applied at fp32 during eviction (scalar engine), with all fp8 operand
scales folded into the BN scale/shift.

Precision: conv1/conv4 (1024->256), conv2 (256->64) and conv5 (256->256)
run fp8e4m3 with perf_mode=DoubleRow (K=256/pass; measured period is
N cols/cycle with LDWEIGHTS hidden, so fewer K-passes is the whole win).
conv3 (64->256) runs bf16 with tap pairing: wout holds a one-row-shifted
copy in partitions 64:127 (written by an SBUF->SBUF DMA, not the scalar
engine), so the dh=-1/dh=0 taps contract together (K=128) and only dh=+1
runs K=64.

Weighting runs as banded matmuls (8 out rows per PSUM bank): for output row
r, out_r[c,w] = sum_dh x_row(r+dh) @ M[r,dh], M built on the host with the
dh=j-4 / dh=j+1 tap pairs stacked in 128 partitions. x2 rows are transposed
to [w,c] blocks by DVE 32x32 stream-transposes overlapping conv4's PE work.

conv3 and conv5 are interleaved per block (staggered by two) so conv5
never waits on an eviction+subtract chain; per-row spatial sums of
relu(bn5(conv5)) leave as [128,32] DMAs per co half and the host finishes
the mean and the two tiny FCs (the flip makes row sums order-invariant, so
no unflip is needed).

Measured on trn2 (NTFF profile, core 0): ~207 us/core in the device's
current ~2.38 GHz power state (vs ~282 us for the previous 40-row PADW=66
version on the same device; ~235 us was that version's full-clock time).
The tensor engine runs at the 1 col/cycle matmul roofline for ~189 us of
the span; the rest is fixed framework prologue (~7 us), warmup/DMA gating
(~5 us), and teardown (~10 us). Rel err ~4.7e-03 vs the fp32 reference
(tolerance 2e-2).
"""

import ml_dtypes
import numpy as np

import concourse.bacc as bacc
import concourse.bass as bass
import concourse.mybir as mybir
from concourse import tile
from concourse.alu_op_type import AluOpType
from concourse.bass_utils import run_bass_kernel_spmd

F32 = mybir.dt.float32
BF16 = mybir.dt.bfloat16
FP8 = mybir.dt.float8e4
NPBF = ml_dtypes.bfloat16
NPF8 = ml_dtypes.float8_e4m3
SX = 8.0    # fp8 input scale (conv1/conv4 activations)
SW = 256.0  # fp8 weight scale (w1/w4); 1/(SX*SW) folded into BN scale
PADW, PADH = 65, 48
PADN = PADW * PADH  # 3120
CL, CH = 1, 65      # valid col window within a padded row
B1 = [(4, 7), (11, 7), (18, 7), (25, 7), (32, 7), (39, 4)]  # conv1: 39 rows
B2 = [(4, 7), (11, 7), (18, 7), (25, 7), (32, 7), (39, 3)]  # conv2: 38 rows
BW = [(4, 8), (12, 8), (20, 8), (28, 8), (36, 2)]           # weighting: 34
B3 = [(4, 7), (11, 7), (18, 7), (25, 7), (32, 5)]           # conv3/4: 33
B5 = [(4, 7), (11, 7), (18, 7), (25, 7), (32, 4)]           # conv5: 32
SHIFTS = [(dh, dw) for dh in (-1, 0, 1) for dw in (-1, 0, 1)]
BN_EPS = 1e-5
H = W = 64
WTROWS = 34         # weighting output rows (slab rows 4..37)
XTN = PADH * 64     # 3072: xT free size (48 row-blocks of 64)
MBN = WTROWS * 5 * 64  # 10880: banded matrices, 5 dh-groups per out row
XK0R = 13           # starter tile rows (covers conv1 block b0)
XK0N = XK0R * PADW  # 845
XK0P = 848          # padded per-plane stride (16B-aligned for DoubleRow)
WARMN = 20          # PE warmup matmuls (cover the gating DMAs + clock ramp)
WARMC = 256         # warmup matmul free size (fine-grained end alignment)


# ---------------------------------------------------------------- program --

def _build_program(probes=False):
    nc = bacc.Bacc("TRN2", target_bir_lowering=False, debug=False)

    lk = nc.dram_tensor("lk", [4, 128, 2 * PADN], FP8, kind="ExternalInput")[:]
    lk0s = nc.dram_tensor("lk0s", [128, 2 * XK0P], FP8, kind="ExternalInput")[:]
    ln = nc.dram_tensor("ln", [4, 128, 2 * PADN], FP8, kind="ExternalInput")[:]
    mb_d = nc.dram_tensor("mb", [128, MBN], BF16, kind="ExternalInput")[:]
    w1 = nc.dram_tensor("w1", [4, 128, 2 * 2304], FP8, kind="ExternalInput")[:]
    w1s_d = nc.dram_tensor("w1s", [128, 2 * 128], FP8, kind="ExternalInput")[:]
    w2 = nc.dram_tensor("w2", [128, 2 * 576], FP8, kind="ExternalInput")[:]
    w3 = nc.dram_tensor("w3", [128, 6 * 256], BF16, kind="ExternalInput")[:]
    w4 = nc.dram_tensor("w4", [4, 128, 2 * 2304], FP8, kind="ExternalInput")[:]
    w5 = nc.dram_tensor("w5", [128, 2 * 2304], FP8, kind="ExternalInput")[:]
    bnp_d = nc.dram_tensor("bnp", [128, 18], F32, kind="ExternalInput")[:]
    osum = nc.dram_tensor("osum", [2, 128, 32], F32, kind="ExternalOutput")[:]

    # bnp columns: [c1sc0,c1sc1,c1sh0,c1sh1, c2sc,c2sh, c3sc0,c3sc1,c3sh0,
    #               c3sh1, c4sc0,c4sc1,c4sh0,c4sh1, c5sc0,c5sc1,c5sh0,c5sh1]
    C1SC, C1SH, C2SC, C2SH = 0, 2, 4, 5
    C3SC, C3SH, C4SC, C4SH, C5SC, C5SH = 6, 8, 10, 12, 14, 16

    with tile.TileContext(nc) as tc:
        with (
            tc.tile_pool(name="sb", bufs=1) as sb,
            tc.tile_pool(name="cps", bufs=6, space="PSUM") as cps,
            tc.tile_pool(name="aps", bufs=2, space="PSUM") as aps,
        ):
            # big resident buffers: full 1024-ch fp8 slab (lk, then reused
            # for ln ktiles 2..3) and full 1024-ch fp8 conv weights (w1,
            # then w4); each ktile chunk is [128, (g=2, n)] for DoubleRow
            xbig = sb.tile([128, 4 * 2 * PADN], FP8, name="xbig", tag="xbig")
            wbig = sb.tile([128, 4 * 2 * 2304], FP8, name="wbig", tag="wbig")
            slabs = [sb.tile([128, 2 * PADN], FP8, name=f"slab{i}", tag=f"slab{i}") for i in range(2)]
            w5sb = sb.tile([128, 2 * 2304], FP8, name="w5sb", tag="w5sb")
            c18 = sb.tile([128, 2 * PADN], FP8, name="c18", tag="c18")
            d8 = sb.tile([128, 2 * PADN], FP8, name="d8", tag="d8")
            c1 = [sb.tile([128, PADN], BF16, name=f"c1_{i}", tag=f"c1_{i}") for i in range(2)]
            ybuf = [sb.tile([128, PADN], BF16, name=f"y{i}", tag=f"y{i}") for i in range(2)]
            x2 = sb.tile([64, PADN], BF16, name="x2", tag="x2")
            xT = sb.tile([128, XTN], BF16, name="xT", tag="xT")
            mbsb = sb.tile([128, MBN], BF16, name="mbsb", tag="mbsb")
            # wout: weighting result in partitions 0:64; partitions 64:128
            # hold a one-row-shifted copy (SBUF->SBUF DMA) so conv3
            # contracts the dh=-1 and dh=0 taps in one K=128 matmul
            wout = sb.tile([128, PADN], BF16, name="wout", tag="wout")
            w2sb = sb.tile([128, 2 * 576], FP8, name="w2sb", tag="w2sb")
            w3sb = sb.tile([128, 6 * 256], BF16, name="w3sb", tag="w3sb")
            c5b = sb.tile([128, 32 * 64], BF16, name="c5b", tag="c5b")
            bnp = sb.tile([128, 18], F32, name="bnp", tag="bnp")
            sums = sb.tile([128, 64], F32, name="sums", tag="sums")
            wrm = sb.tile([128, 512], BF16, name="wrm", tag="wrm")
            # starter tiles: lk ktile-0 rows 0..12 (all block-b0 reads) and
            # w1 ktile-0 si-0, so conv1's first matmuls gate on tiny DMAs
            xk0 = sb.tile([128, 2 * XK0P], FP8, name="xk0", tag="xk0")
            w1s = sb.tile([128, 2 * 128], FP8, name="w1s", tag="w1s")

            def r3(ap):  # [P, n*65] -> [P, n, 65]
                return ap.rearrange("p (r c) -> p r c", c=PADW)

            def xflat(ti):  # xbig ktile chunk (flat, for DMA)
                return xbig[:, ti * 2 * PADN : (ti + 1) * 2 * PADN]

            def xch(ti):  # same chunk as [128, 2, PADN] DoubleRow view
                return xflat(ti).rearrange("p (g n) -> p g n", g=2)

            def wflat(ti):
                return wbig[:, ti * 2 * 2304 : (ti + 1) * 2 * 2304]

            def wch(ti):  # [128, 2, 2304]
                return wflat(ti).rearrange("p (g n) -> p g n", g=2)

            # gating starters ride the otherwise-idle Sync queue; the bulk
            # streams go on GpSimd in consumption order
            nc.sync.dma_start(xk0[:], lk0s)
            nc.sync.dma_start(w1s[:], w1s_d)
            nc.gpsimd.dma_start(xflat(0), lk[0])
            nc.gpsimd.dma_start(wflat(0), w1[0])
            for ti in range(1, 4):
                nc.gpsimd.dma_start(xflat(ti), lk[ti])
                nc.gpsimd.dma_start(wflat(ti), w1[ti])
            nc.gpsimd.dma_start(w2sb[:], w2)
            nc.gpsimd.dma_start(slabs[0][:], ln[0])
            nc.gpsimd.dma_start(slabs[1][:], ln[1])
            nc.gpsimd.dma_start(w3sb[:], w3)
            nc.gpsimd.dma_start(w5sb[:], w5)
            nc.gpsimd.dma_start(mbsb[:], mb_d)
            nc.gpsimd.dma_start(bnp[:], bnp_d)

            # memsets: wrm gates the warmup so it goes first. c18/d8/wout
            # need full zeroing (row 3 and every row's shared pad column
            # are read as conv zero-padding); xT rows 0..3 feed the
            # weighting; c1/ybuf are fully written before any read.
            nc.vector.memset(wrm[:], 0.0)
            nc.vector.memset(c18[:], 0.0)
            nc.vector.memset(xT[:], 0.0)
            nc.vector.memset(wout[0:64, :], 0.0)
            nc.vector.memset(d8[:], 0.0)

            DR = mybir.MatmulPerfMode.DoubleRow

            # PE warmup on zeroed scratch while the gating DMAs land: ramps
            # the tensor engine out of its cold p-state
            pwu = aps.tile([64, WARMC], F32, name="pw", tag="pw")
            for i in range(WARMN):
                nc.tensor.matmul(
                    out=pwu[:, :WARMC], lhsT=wrm[:, 0:64], rhs=wrm[:, 0:WARMC],
                    start=(i == 0), stop=(i == WARMN - 1),
                )

            def conv_ko(rhs_of, lhsT_of, nkt, m, emit, blocks, dr=False,
                        first_rhs=None, first_lhsT=None):
                """ktile-outer: all block PSUMs live; streams rhs once.
                first_rhs substitutes the rhs for (ti=0, block 0) and
                first_lhsT the weights for (ti=0, si=0), so the first
                matmuls gate on small starter DMAs."""
                psts = [cps.tile([m, nr * PADW], F32, name="cp", tag="cp")
                        for _, nr in blocks]
                for ti in range(nkt):
                    rhs = rhs_of(ti)
                    for bi, (r0, nr) in enumerate(blocks):
                        rr = first_rhs if (ti == 0 and bi == 0
                                           and first_rhs is not None) else rhs
                        for si, (dh, dw) in enumerate(SHIFTS):
                            lh = (first_lhsT if (ti == 0 and si == 0
                                                 and first_lhsT is not None)
                                  else lhsT_of(ti, si))
                            o = (r0 + dh) * PADW + dw
                            nc.tensor.matmul(
                                out=psts[bi][:, : nr * PADW],
                                lhsT=lh,
                                rhs=rr[:, :, o : o + nr * PADW] if dr
                                else rr[:, o : o + nr * PADW],
                                start=(ti == 0 and si == 0),
                                stop=(ti == nkt - 1 and si == 8),
                                perf_mode=DR if dr else None,
                            )
                for bi, (r0, nr) in enumerate(blocks):
                    emit(bi, r0, nr, psts[bi])

            def mk_evict(dst2d, sc_col, sh_col, m=128):
                def emit(bi, r0, nr, pst):
                    nc.scalar.activation(
                        out=r3(dst2d)[:, r0 : r0 + nr, CL:CH],
                        in_=r3(pst[:m, : nr * PADW])[:, :, CL:CH],
                        func=mybir.ActivationFunctionType.Relu,
                        scale=bnp[:m, sc_col : sc_col + 1],
                        bias=bnp[:m, sh_col : sh_col + 1],
                    )
                return emit

            # ---- conv1: 1024 -> 256 (fp8 DoubleRow, K=256/matmul) --------
            conv_ko(xch,
                    lambda ti, si: wch(ti)[:, :, si * 256 : si * 256 + 128],
                    4, 128, mk_evict(c18[:, 0:PADN], C1SC, C1SH), B1, dr=True,
                    first_rhs=xk0[:].rearrange("p (g n) -> p g n", g=2),
                    first_lhsT=w1s[:].rearrange("p (g n) -> p g n", g=2))
            conv_ko(xch,
                    lambda ti, si: wch(ti)[:, :, si * 256 + 128 : si * 256 + 256],
                    4, 128, mk_evict(c18[:, PADN : 2 * PADN], C1SC + 1, C1SH + 1),
                    B1, dr=True)

            # conv4 inputs: ln ktiles 2..3 and w4 reuse xbig/wbig as soon as
            # conv1 releases them
            for ti in range(4):
                if ti >= 2:
                    nc.gpsimd.dma_start(xflat(ti), ln[ti])
                nc.gpsimd.dma_start(wflat(ti), w4[ti])

            # ---- conv2: 256 -> 64 (fp8 DoubleRow), rhs = c18, out -> x2 ---
            conv_ko(lambda ti: c18[:].rearrange("p (g n) -> p g n", g=2),
                    lambda ti, si: w2sb[:].rearrange("p (g n) -> p g n", g=2)
                    [:, :, si * 64 : si * 64 + 64],
                    1, 64, mk_evict(x2[:], C2SC, C2SH, m=64), B2, dr=True)

            # ---- xT: DVE stream-transpose x2 rows 4..41 to [w, c] blocks
            # (SBUF->SBUF, 32x32 blocks with swapped block positions; runs
            # on DVE so it overlaps conv4's PE work)
            for j in range(4, 42):
                for bi in range(2):
                    for bj in range(2):
                        nc.vector.transpose(
                            out=xT[32 * bj : 32 * bj + 32,
                                   j * 64 + 32 * bi : j * 64 + 32 * bi + 32],
                            in_=r3(x2[:])[32 * bi : 32 * bi + 32, j,
                                          CL + 32 * bj : CL + 32 + 32 * bj],
                        )
            # +5-row-shifted duplicate in partitions 64:127 (pairs the
            # dh=j-4 lower tap with dh=j+1 upper tap in one contraction)
            nc.gpsimd.dma_start(
                xT[64:128, 0 : (PADH - 5) * 64], xT[0:64, 5 * 64 : XTN]
            )

            # ---- conv4 (y branch): 1024 -> 256 from resident buffers ------
            def ln_rhs(ti):
                if ti < 2:
                    return slabs[ti][:].rearrange("p (g n) -> p g n", g=2)
                return xch(ti)

            for co in range(2):
                conv_ko(ln_rhs,
                        lambda ti, si, _co=co: wch(ti)[:, :, si * 256 + _co * 128 :
                                                       si * 256 + _co * 128 + 128],
                        4, 128, mk_evict(ybuf[co][:], C4SC + co, C4SH + co),
                        B3, dr=True)

            # ---- weighting: banded matmuls, up to 8 out rows per PSUM bank;
            # main evict on scalar, the row-shifted duplicate via DMA -------
            for (r0, nr) in BW:
                pw = aps.tile([64, nr * 64], F32, name="pw", tag="pw")
                for rr in range(nr):
                    r = r0 + rr
                    for j in range(5):
                        nc.tensor.matmul(
                            out=pw[:, rr * 64 : rr * 64 + 64],
                            lhsT=xT[:, (r + j - 4) * 64 : (r + j - 4) * 64 + 64],
                            rhs=mbsb[:, ((r - 4) * 5 + j) * 64 :
                                     ((r - 4) * 5 + j) * 64 + 64],
                            start=(j == 0), stop=(j == 4),
                        )
                nc.scalar.activation(
                    out=r3(wout[0:64, 0:PADN])[:, r0 : r0 + nr, CL:CH],
                    in_=pw.rearrange("p (r c) -> p r c", c=64)[:, :, :],
                    func=mybir.ActivationFunctionType.Copy,
                )
                nc.sync.dma_start(
                    wout[64:128, (r0 - 1) * PADW : (r0 - 1 + nr) * PADW],
                    wout[0:64, r0 * PADW : (r0 + nr) * PADW],
                )

            # ---- conv3 (64 -> 256, 6 tap-groups: 3 paired K=128 + 3 single
            # K=64, then x3 - y into d8) interleaved with conv5 blocks ------
            def c3_block(bi, co):
                r0, nr = B3[bi]
                pst = cps.tile([128, nr * PADW], F32, name="cp", tag="cp")
                for gi in range(6):
                    dh = -1 if gi < 3 else 1
                    dw = gi % 3 - 1
                    o = (r0 + dh) * PADW + dw
                    kp = 128 if gi < 3 else 64
                    nc.tensor.matmul(
                        out=pst[:, : nr * PADW],
                        lhsT=w3sb[0:kp, gi * 256 + co * 128 :
                                  gi * 256 + co * 128 + 128],
                        rhs=wout[0:kp, o : o + nr * PADW],
                        start=(gi == 0), stop=(gi == 5),
                    )
                nc.scalar.activation(
                    out=r3(c1[co][:])[:, r0 : r0 + nr, CL:CH],
                    in_=r3(pst[:, : nr * PADW])[:, :, CL:CH],
                    func=mybir.ActivationFunctionType.Relu,
                    scale=bnp[:, C3SC + co : C3SC + co + 1],
                    bias=bnp[:, C3SH + co : C3SH + co + 1],
                )
                # data columns only: d8's pad columns keep their memset
                # zeros (they are read as conv5's dw zero-padding)
                nc.vector.tensor_tensor(
                    out=r3(d8[:, co * PADN : (co + 1) * PADN])
                    [:, r0 : r0 + nr, CL:CH],
                    in0=r3(c1[co][:])[:, r0 : r0 + nr, CL:CH],
                    in1=r3(ybuf[co][:])[:, r0 : r0 + nr, CL:CH],
                    op=AluOpType.subtract,
                )

            c5b3 = c5b.rearrange("p (r c) -> p r c", c=64)

            def c5_block(bi, co):
                r0, nr = B5[bi]
                pst = aps.tile([128, nr * PADW], F32, name="pw", tag="pw")
                for si, (dh, dw) in enumerate(SHIFTS):
                    o = (r0 + dh) * PADW + dw
                    nc.tensor.matmul(
                        out=pst[:, : nr * PADW],
                        lhsT=w5sb[:].rearrange("p (g n) -> p g n", g=2)
                        [:, :, si * 256 + co * 128 : si * 256 + co * 128 + 128],
                        rhs=d8[:].rearrange("p (g n) -> p g n", g=2)
                        [:, :, o : o + nr * PADW],
                        start=(si == 0), stop=(si == 8),
                        perf_mode=DR,
                    )
                nc.scalar.activation(
                    out=c5b3[:, r0 - 4 : r0 - 4 + nr, :],
                    in_=r3(pst[:, : nr * PADW])[:, :, CL:CH],
                    func=mybir.ActivationFunctionType.Relu,
                    scale=bnp[:, C5SC + co : C5SC + co + 1],
                    bias=bnp[:, C5SH + co : C5SH + co + 1],
                )
                nc.vector.tensor_reduce(
                    out=sums[:, co * 32 + r0 - 4 : co * 32 + r0 - 4 + nr],
                    in_=c5b3[:, r0 - 4 : r0 - 4 + nr, :],
                    axis=mybir.AxisListType.X, op=AluOpType.add,
                )

            # conv5 block b reads d8 rows r0-1..r0+nr, i.e. needs conv3
            # blocks b-1..b+1 of BOTH co halves; stagger by two so the PE
            # never waits on an eviction+subtract chain
            for bi in range(3):
                c3_block(bi, 0)
                c3_block(bi, 1)
            for bi in range(3, 5):
                c5_block(bi - 3, 0)
                c5_block(bi - 3, 1)
                c3_block(bi, 0)
                c3_block(bi, 1)
            for bi in range(2, 5):
                c5_block(bi, 0)
                c5_block(bi, 1)

            # output: per-row sums, one DMA per co half
            for co in range(2):
                nc.gpsimd.dma_start(
                    osum[co], sums[:, co * 32 : co * 32 + 32])

            if probes:
                for nm, t, shp, dt in [
                    ("c18", c18, [128, 2 * PADN], FP8),
                    ("x2", x2, [64, PADN], BF16),
                    ("wout", wout, [128, PADN], BF16),
                    ("y0", ybuf[0], [128, PADN], BF16),
                    ("y1", ybuf[1], [128, PADN], BF16),
                    ("d8", d8, [128, 2 * PADN], FP8),
                    ("c5b", c5b, [128, 32 * 64], BF16),
                    ("xTd", xT, [128, XTN], BF16),
                ]:
                    dd = nc.dram_tensor(f"dbg_{nm}", shp, dt,
                                        kind="ExternalOutput")[:]
                    nc.gpsimd.dma_start(dd, t[:])

    nc.compile()
    return nc


# ------------------------------------------------------------- host side --

def _pad_slab(x_bchw, flip):
    """[1024, 64, 64] -> [4, 128, 2*PADN] fp8 (scaled by SX); slab rows
    4..47 = (optionally H-flipped) image rows 0..43, rows 0..3 zero;
    channel c = kt*256 + g*128 + p."""
    xx = x_bchw[:, ::-1, :] if flip else x_bchw
    out = np.zeros((1024, PADH, PADW), np.float32)
    out[:, 4:48, CL:CH] = xx[:, 0:44, :]
    v = out.reshape(4, 2, 128, PADN).transpose(0, 2, 1, 3)  # [kt, p, g, n]
    v = np.clip(v * SX, -224.0, 224.0)
    return np.ascontiguousarray(v).reshape(4, 128, 2 * PADN).astype(NPF8)


def _fold_bn(bn):
    g, b, m, v = [np.asarray(x, np.float32) for x in bn]
    sc = g / np.sqrt(v + BN_EPS)
    return sc, b - m * sc


def _wt8(w):
    """[256, 1024, 3, 3] -> [4, 128, 2*2304] fp8 DoubleRow lhsT (scaled by
    SW): [kt, p, (g, si*256 + co)]."""
    v = w.reshape(256, 4, 2, 128, 9).transpose(1, 3, 2, 4, 0)  # [kt,p,g,9,co]
    v = np.clip(v * SW, -224.0, 224.0)
    return np.ascontiguousarray(v).reshape(4, 128, 2 * 2304).astype(NPF8)


def _w3p(w):
    """[256, 64, 3, 3] -> [128, 6*256] bf16: tap-pair lhsT for conv3.
    Groups 0..2: rows 0:64 = (dh=-1, dw=g-1), rows 64:128 = (dh=0, dw=g-1);
    groups 3..5: rows 0:64 = (dh=+1, dw=g-3-1), rows 64:128 unused."""
    out = np.zeros((128, 6, 256), np.float32)
    for g in range(3):
        out[0:64, g] = w[:, :, 0, g].T
        out[64:128, g] = w[:, :, 1, g].T
        out[0:64, g + 3] = w[:, :, 2, g].T
    return out.reshape(128, 6 * 256).astype(NPBF)


def _wt8_k256(w):
    """[Co, 256, 3, 3] -> [128, 2*9*Co] fp8 DoubleRow lhsT (scaled by SW):
    [p, (g, si*Co + co)] for a single 256-deep contraction tile."""
    co = w.shape[0]
    v = w.reshape(co, 2, 128, 9).transpose(2, 1, 3, 0)  # [p, g, 9, co]
    v = np.clip(v * SW, -224.0, 224.0)
    return np.ascontiguousarray(v).reshape(128, 2 * 9 * co).astype(NPF8)


def _band_mats(att, flip):
    """att: [64, 64, 81] -> mb [128, MBN] bf16 banded pair matrices for
    local output rows 0..33 (slab rows 4..37).

    M[i, kh][w_in, w] = att_local[i, w, kh*9 + (w_in-w)+4]; group j pairs
    kh=j (lower partitions, dh=j-4) with kh=j+5 (upper, dh=j+1)."""
    A4 = np.asarray(att, np.float32).reshape(64, 64, 9, 9)
    if flip:
        A4 = A4[::-1, :, ::-1, :]
    A = A4[0:WTROWS]  # [rows, w, kh, kw]
    M = np.zeros((WTROWS, 9, 64, 64), np.float32)
    idx = np.arange(64)
    for d in range(-4, 5):
        v = idx[(idx + d >= 0) & (idx + d < 64)]
        # A[:, v, :, d+4] puts the fancy-indexed axis first: [len(v), rows, 9]
        M[:, :, v + d, v] = A[:, v, :, d + 4].transpose(1, 2, 0)
    mb = np.zeros((WTROWS, 5, 128, 64), np.float32)
    mb[:, 0:4, 0:64, :] = M[:, 0:4]
    mb[:, 0:4, 64:128, :] = M[:, 5:9]
    mb[:, 4, 0:64, :] = M[:, 4]
    return np.ascontiguousarray(mb.transpose(2, 0, 1, 3)).reshape(
        128, MBN).astype(NPBF)


def prep_core_inputs(inputs, core):
    b, flip = core // 2, bool(core % 2)
    lk = _pad_slab(np.asarray(inputs["low_key"][b], np.float32), flip)
    ln = _pad_slab(np.asarray(inputs["low_nonkey"][b], np.float32), flip)
    mb = _band_mats(np.asarray(inputs["local_atten"][b], np.float32), flip)

    def wf(name):  # conv weights, dh-flipped for odd cores
        w = np.asarray(inputs[name], np.float32)
        return w[:, :, ::-1, :] if flip else w

    bnp = np.zeros((128, 18), np.float32)
    for i, (name, cols) in enumerate(
        [("bn1", (0, 2)), ("bn2", (4, 5)), ("bn3", (6, 8)),
         ("bn4", (10, 12)), ("bn5", (14, 16))]
    ):
        sc, sh = _fold_bn(np.asarray(inputs[name]))
        if name == "bn1":
            # undo fp8 operand scaling, then scale the fp8 output by SX
            # (ReLU commutes with the positive scale)
            sc = sc * (SX / (SX * SW))
            sh = sh * SX
        elif name == "bn2":
            sc = sc / (SX * SW)  # c18 carries SX; w2 carries SW
        elif name == "bn4":
            sc = sc / (SX * SW)
        elif name == "bn5":
            sc = sc / SW  # d8 is unscaled; w5 carries SW
        nco = sc.shape[0]
        if nco == 256:
            bnp[:, cols[0]] = sc[:128]
            bnp[:, cols[0] + 1] = sc[128:]
            bnp[:, cols[1]] = sh[:128]
            bnp[:, cols[1] + 1] = sh[128:]
        else:
            bnp[:64, cols[0]] = sc
            bnp[:64, cols[1]] = sh

    w1p = _wt8(wf("w1"))
    # starter: ktile-0, si-0 weight tile [p, (g, m)]
    w1s = np.ascontiguousarray(
        w1p[0].reshape(128, 2, 2304)[:, :, 0:128]).reshape(128, 256)
    # starter: lk ktile-0 rows 0..12 with 848-wide (16B-aligned) planes,
    # packed contiguously so the gating DMA is a flat copy
    lk0s = np.zeros((128, 2, XK0P), np.float32)
    lk0s[:, :, 0:XK0N] = lk[0].astype(np.float32).reshape(128, 2, PADN)[:, :, 0:XK0N]
    lk0s = lk0s.reshape(128, 2 * XK0P).astype(NPF8)
    return {
        "lk": lk, "ln": ln, "mb": mb, "lk0s": lk0s,
        "w1": w1p, "w1s": w1s,
        "w2": _wt8_k256(wf("w2")),
        "w3": _w3p(wf("w3")),
        "w4": _wt8(wf("w4")),
        "w5": _wt8_k256(wf("w5")),
        "bnp": bnp,
    }


def postprocess(osums, inputs):
    """osums: list of 8 arrays [2, 128, 32] -> final [4, 1]."""
    mean = np.zeros((4, 256), np.float32)
    for core in range(8):
        b = core // 2
        mean[b] += osums[core].reshape(256, 32).sum(axis=1)
    mean /= float(H * W)
    fw1 = np.asarray(inputs["fw1"], np.float32)
    fb1 = np.asarray(inputs["fb1"], np.float32)
    fw2 = np.asarray(inputs["fw2"], np.float32)
    fb2 = np.asarray(inputs["fb2"], np.float32)
    out = mean @ fw1.T + fb1
    out = out @ fw2.T + fb2
    return out.astype(np.float32)


_prog_cache = {}
LAST = {}


def kernel(**inputs) -> np.ndarray:
    import os, time
    if "nc" not in _prog_cache:
        _prog_cache["nc"] = _build_program(
            probes=bool(os.environ.get("KS_PROBES")))
    nc = _prog_cache["nc"]
    in_maps = [prep_core_inputs(inputs, core) for core in range(8)]
    want_trace = bool(os.environ.get("KS_TRACE"))
    if want_trace:
        try:
            import antenv.axon_hooks  # noqa: F401  (absent in bare envs)
        except ImportError:
            want_trace = False
    t0 = time.time()
    res = run_bass_kernel_spmd(
        nc, in_maps, list(range(8)), trace=want_trace
    )
    LAST["spmd_s"] = time.time() - t0
    LAST["res"] = res
    return postprocess([r["osum"] for r in res.results], inputs)
